# revision 22
# baseline (speedup 1.0000x reference)
"""Causal GQA attention (B=2, T=2048, H=16, KV=4, d=128, rope=32) on 8 trn2 cores.

Sharding: core c handles batch b = c // 4 and kv-head-group g = c % 4
(4 query heads + 1 kv head per core). Wq/Wk/Wv column-sharded, Wo
row-sharded; the Wo all-reduce is done on the host during unshard.

Attention uses an augmented-V trick: PV psum is [q, d+1] with a ones
column on V, so the softmax denominator comes out of the same matmuls
(no separate ones-row reduction). Causal masking is a constant 128x128
triangle multiply on DVE after exp (no mask matmuls). P/V/ot/Wo run in
bf16; scores and projections stay fp32r.
"""

import math
import sys

sys.path.insert(0, "/opt/trn_rl_repo")

import numpy as np

N_CORES = 8
B, T, C = 2, 2048, 2048
NH, NKV, HD = 16, 4, 128
GRP = NH // NKV          # 4 query heads per core
ROPE = 32
QK_GAIN = 6.0
NCH = T // 512           # 4 column chunks of 512
NKT = C // 128           # 16 contraction tiles
NTT = T // 128           # 16 row tiles

_build_cache = {}


# ---------------------------------------------------------------- device code


def _emit(nc, tc, dram, p, mybir):
    R = mybir.dt.float32r
    F = mybir.dt.float32
    BF = mybir.dt.bfloat16
    Exp = mybir.ActivationFunctionType.Exp
    mult = mybir.AluOpType.mult
    add = mybir.AluOpType.add

    (xT, wq, wk, wv, wo, z) = dram
    A = p["psA"]
    Bp = p["psB"]

    qt_all = p["qt"].tile([128, GRP, T], R, tag="qt", bufs=1)
    qt_tiles = [qt_all[:, h, :] for h in range(GRP)]
    kt_tile = p["qt"].tile([128, T], R, tag="kt", bufs=1)
    v_aug = p["qt"].tile([128, NTT, HD + 1], BF, tag="va", bufs=1)  # [keys, d | 1]
    nc.vector.memset(v_aug[:, :, HD], 1.0)
    wq_res = p["wq_res"]
    wo_sb = p["wo_sb"]

    # ---------------- phase 1: QT[h] = (Wq_h)^T x^T, KT, V ----------------
    pending_ropes = []

    def rope_chunk(dst, cch, rot_ps):
        cs = slice(cch * 512, (cch + 1) * 512)
        nc.tensor.matmul(rot_ps, p["rot_sb"][:], dst[0:32, cs],
                         start=True, stop=True)
        t2 = p["rp"].tile([32, 512], F, tag="rp", bufs=2)
        qc = p["rp"].tile([32, 512], F, tag="rp", bufs=2)
        nc.gpsimd.tensor_tensor(qc[:], dst[0:32, cs], p["cos_sb"][:, cs], op=mult)
        nc.vector.tensor_tensor(t2[:], rot_ps, p["sin_sb"][:, cs], op=mult)
        nc.vector.tensor_tensor(dst[0:32, cs], t2[:], qc[:], op=add)

    for cch in range(NCH):
        cs = slice(cch * 512, (cch + 1) * 512)
        qpa = A.tile([128, 1024], F, tag="A", bufs=2)
        qpb = A.tile([128, 1024], F, tag="A", bufs=2)
        q_ps = [qpa[:, 0:512], qpa[:, 512:1024], qpb[:, 0:512], qpb[:, 512:1024]]
        kv = Bp.tile([128, 1024], F, tag="B", bufs=2)   # k | vt halves
        rv = Bp.tile([128, 1024], F, tag="B", bufs=2)   # rope rotation scratch
        for kg in range(NKT // 2):
            xt2 = p["xs"].tile([128, 2, 512], R, tag="xs", bufs=4)
            nc.sync.dma_start(out=xt2, in_=xT[:, 2 * kg:2 * kg + 2, cs])
            for i in range(2):
                kt = 2 * kg + i
                xt = xt2[:, i, :]
                st, sp = (kt == 0), (kt == NKT - 1)
                for m in range(GRP):
                    nc.tensor.matmul(q_ps[m], wq_res[:, kt, m * 128:(m + 1) * 128],
                                     xt, start=st, stop=sp)
                nc.tensor.matmul(kv[:, 0:512], p["wk_sb"][:, kt, :], xt,
                                 start=st, stop=sp)
                nc.tensor.matmul(kv[:, 512:1024], p["wv_sb"][:, kt, :], xt,
                                 start=st, stop=sp)
            if pending_ropes:
                pending_ropes.pop(0)()
        # copies out: kt/vt first so the V transposes can start early
        nc.scalar.copy(kt_tile[:, cs], kv[:, 0:512])
        vt_sb = p["vts"].tile([128, 512], F, tag="vts", bufs=2)
        nc.scalar.copy(vt_sb[:], kv[:, 512:1024])
        for pr in range(2):
            nc.scalar.copy(qt_all[:, 2 * pr:2 * pr + 2, cs],
                           (qpa if pr == 0 else qpb)[:].rearrange(
                               "p (m t) -> p m t", m=2))
        # VT chunk -> PE transpose per 128-tile -> V natural (bf16)
        for s in range(4):
            jt = cch * 4 + s
            vtr = kv[:, s * 128:(s + 1) * 128]
            nc.tensor.transpose(vtr, vt_sb[:, s * 128:(s + 1) * 128],
                                p["ident_f"][:])
            nc.vector.tensor_copy(v_aug[:, jt, 0:HD], vtr)
        # ropes run interleaved into the next chunk's kg loop (rv slots are
        # a dedicated bank pair, so no contention with the projections)
        for idx, dst in enumerate([kt_tile] + qt_tiles):
            slot = rv[0:32, (idx % 2) * 512:(idx % 2) * 512 + 512]
            pending_ropes.append(
                (lambda d=dst, c=cch, s=slot: rope_chunk(d, c, s)))

    # remaining ropes must land before attention: their psum scratch (rv)
    # gets recycled as PV accumulators once attention starts
    while pending_ropes:
        pending_ropes.pop(0)()

    # ------- phase 2+3: attention per (chunk, head), then Z for that chunk --
    offs = [0, 256, 512, 768]

    def attention_chunk(cch):
        cs0 = cch * 512
        jmax = 4 * cch + 4
        pvts = {}

        # one accumulating region per psum bank (the hw supports a single
        # open accumulation group per bank): qtile m lives in tile m//2,
        # bank m%2, columns [512*(m%2), 512*(m%2)+129)
        def reg(h, m):
            pvt = pvts[h][m // 2]
            off = 512 * (m % 2)
            return pvt, off

        def emit_pv(h, jt0, pt):
            if h not in pvts:
                pvts[h] = (Bp.tile([128, 1024], F, name=f"pv{h}a", tag="B", bufs=2),
                           Bp.tile([128, 1024], F, name=f"pv{h}b", tag="B", bufs=2))
            for s2 in range(2):
                jt = jt0 + s2
                for m in range(GRP):
                    if 4 * cch + m < jt:
                        continue
                    pvt, off = reg(h, m)
                    nc.tensor.matmul(pvt[:, off:off + HD + 1],
                                     pt[:, 512 * s2 + m * 128:512 * s2 + (m + 1) * 128],
                                     v_aug[:, jt, :],
                                     start=(jt == 0), stop=(jt == 4 * cch + m))

        def finalize_a(h, m):
            pvt, off = reg(h, m)
            rcp = p["rcp"].tile([128, 1], F, tag="rcp", bufs=4)
            nc.vector.reciprocal_approx_fast(rcp[:], pvt[:, off + HD:off + HD + 1])
            otq = p["otq"].tile([128, 128], F, tag="otq", bufs=4)
            nc.vector.tensor_scalar_mul(otq[:], pvt[:, off:off + HD], rcp[:])
            return otq

        def finalize_b(h, m, otq):
            pvt, off = reg(h, m)
            nc.tensor.transpose(pvt[:, off:off + HD], otq[:], p["ident_f"][:])
            dst = p["ot_sb"][h][:, cs0 + m * 128:cs0 + (m + 1) * 128]
            if m % 2 == 0:
                nc.scalar.copy(dst, pvt[:, off:off + HD])
            else:
                nc.vector.tensor_copy(dst, pvt[:, off:off + HD])

        fin_pend = []

        def drain_one():
            ph, pj, ppt = pend.pop(0)
            # before a new head's PV starts in the shared psum tiles, all
            # lingering writes to the previous head's regions must be emitted
            if ph not in pvts:
                while fin_pend:
                    finalize_b(*fin_pend.pop(0))
            emit_pv(ph, pj, ppt)
            newfin = []
            for m in range(GRP):
                if pj <= 4 * cch + m <= pj + 1:
                    otq = finalize_a(ph, m)
                    newfin.append((ph, m, otq))
            # flush transposes that have aged two drains (Pool ts certainly
            # done; avoids PE stalls and Pool head-of-line blocking)
            while len(fin_pend) > 2:
                finalize_b(*fin_pend.pop(0))
            fin_pend.extend(newfin)

        pend = []
        for h in range(GRP):
            for jt0 in range(0, jmax, 2):
                if len(pend) == 3:
                    drain_one()
                stp = A.tile([128, 1024], F, tag="A", bufs=2)
                for s2 in range(2):
                    jt = jt0 + s2
                    sdiag = jt - 4 * cch
                    qstart = 0 if sdiag < 0 else min(128 * sdiag, 256)
                    nc.tensor.matmul(stp[:, 512 * s2 + qstart:512 * (s2 + 1)],
                                     kt_tile[:, jt * 128:(jt + 1) * 128],
                                     qt_tiles[h][:, cs0 + qstart:cs0 + 512],
                                     start=True, stop=True)
                pt = p["pt"].tile([128, 1024], BF, tag="pt", bufs=4)
                etrim = 256 if (jt0 - 4 * cch) >= 2 else 0
                nc.scalar.activation(pt[:, etrim:1024], stp[:, etrim:1024], Exp)
                for s2 in range(2):
                    sdiag = jt0 + s2 - 4 * cch
                    if sdiag >= 0:
                        c0 = 512 * s2 + 128 * sdiag
                        nc.vector.tensor_tensor(pt[:, c0:c0 + 128],
                                                pt[:, c0:c0 + 128],
                                                p["tri_sb"][:], op=mult)
                pend.append((h, jt0, pt))
        while pend:
            drain_one()
        while fin_pend:
            finalize_b(*fin_pend.pop(0))

    for cch in range(NCH):
        attention_chunk(cch)
        # Z rows for this chunk: Z[m,:] needs OT[:, chunk] from all 4 heads.
        for m in range(4 * cch, 4 * cch + 4):
            pool = A if (m % 2 == 0) else Bp
            tag = "A" if (m % 2 == 0) else "B"
            zlo = pool.tile([128, 1024], F, tag=tag, bufs=2)
            zhi = pool.tile([128, 1024], F, tag=tag, bufs=2)
            zq = [zlo[:, 0:512], zlo[:, 512:1024], zhi[:, 0:512], zhi[:, 512:1024]]
            for h in range(GRP):
                lhs = p["ot_sb"][h][:, m * 128:(m + 1) * 128]
                for nchk in range(NCH):
                    nc.tensor.matmul(zq[nchk], lhs,
                                     wo_sb[:, h, nchk * 512:(nchk + 1) * 512],
                                     start=(h == 0), stop=(h == GRP - 1))
            zs_lo = p["zs"].tile([128, 1024], F, tag="zs", bufs=4)
            zs_hi = p["zs"].tile([128, 1024], F, tag="zs", bufs=4)
            nc.scalar.copy(zs_lo[:], zlo[:])
            nc.vector.tensor_copy(zs_hi[:], zhi[:])
            nc.sync.dma_start(out=z[m * 128:(m + 1) * 128, 0:1024], in_=zs_lo)
            nc.sync.dma_start(out=z[m * 128:(m + 1) * 128, 1024:2048], in_=zs_hi)


def _build(loop_iters=None):
    if loop_iters in _build_cache:
        return _build_cache[loop_iters]
    import concourse.bacc as bacc
    import concourse.tile as tile
    import concourse.mybir as mybir

    R = mybir.dt.float32r
    F = mybir.dt.float32
    BF = mybir.dt.bfloat16

    nc = bacc.Bacc("TRN2", target_bir_lowering=False, debug=False, num_devices=N_CORES)
    xT = nc.dram_tensor("xt", [128, NKT, T], R, kind="ExternalInput").ap()
    wq = nc.dram_tensor("wq", [128, NKT, GRP * HD], R, kind="ExternalInput").ap()
    wk = nc.dram_tensor("wk", [C, HD], R, kind="ExternalInput").ap()
    wv = nc.dram_tensor("wv", [C, HD], R, kind="ExternalInput").ap()
    wo = nc.dram_tensor("wo", [GRP * HD, C], BF, kind="ExternalInput").ap()
    cosd = nc.dram_tensor("cosd", [ROPE, T], F, kind="ExternalInput").ap()
    sind = nc.dram_tensor("sind", [ROPE, T], F, kind="ExternalInput").ap()
    rotd = nc.dram_tensor("rotd", [ROPE, ROPE], R, kind="ExternalInput").ap()
    trid = nc.dram_tensor("trid", [128, 128], BF, kind="ExternalInput").ap()
    identfd = nc.dram_tensor("identfd", [128, 128], F, kind="ExternalInput").ap()
    z = nc.dram_tensor("z", [T, C], F, kind="ExternalOutput").ap()
    dram = (xT, wq, wk, wv, wo, z)

    with tile.TileContext(nc) as tc:
        with tc.tile_pool(name="consts", bufs=1) as consts, \
             tc.tile_pool(name="qt", bufs=1) as qtp, \
             tc.tile_pool(name="xs", bufs=1) as xs, \
             tc.tile_pool(name="vts", bufs=1) as vts, \
             tc.tile_pool(name="rp", bufs=1) as rp, \
             tc.tile_pool(name="pt", bufs=1) as ptp, \
             tc.tile_pool(name="rcp", bufs=1) as rcp, \
             tc.tile_pool(name="otq", bufs=1) as otq, \
             tc.tile_pool(name="ot", bufs=1) as otp, \
             tc.tile_pool(name="zs", bufs=1) as zs, \
             tc.tile_pool(name="psA", bufs=1, space="PSUM") as psA, \
             tc.tile_pool(name="psB", bufs=1, space="PSUM") as psB:

            p = {
                "qt": qtp, "xs": xs, "vts": vts, "rp": rp,
                "pt": ptp, "rcp": rcp, "otq": otq, "zs": zs,
                "psA": psA, "psB": psB,
            }

            # constants, loaded once (k/v slices interleaved so the first
            # contraction tiles land early)
            wk_sb = consts.tile([128, NKT, HD], R)
            wv_sb = consts.tile([128, NKT, HD], R)
            wk_r = wk.rearrange("(k p) m -> p k m", p=128)
            wv_r = wv.rearrange("(k p) m -> p k m", p=128)
            for i in range(4):
                sl = slice(4 * i, 4 * i + 4)
                nc.gpsimd.dma_start(out=wk_sb[:, sl, :], in_=wk_r[:, sl, :])
                nc.gpsimd.dma_start(out=wv_sb[:, sl, :], in_=wv_r[:, sl, :])
            cos_sb = consts.tile([ROPE, T], F)
            nc.gpsimd.dma_start(out=cos_sb, in_=cosd)
            sin_sb = consts.tile([ROPE, T], F)
            nc.gpsimd.dma_start(out=sin_sb, in_=sind)
            rot_sb = consts.tile([ROPE, ROPE], R)
            nc.gpsimd.dma_start(out=rot_sb, in_=rotd)
            tri_sb = consts.tile([128, 128], BF)
            nc.scalar.dma_start(out=tri_sb, in_=trid)
            ident_f = consts.tile([128, 128], F)
            nc.scalar.dma_start(out=ident_f, in_=identfd)
            wq_res = consts.tile([128, NKT, GRP * HD], R)
            for i in range(4):
                nc.scalar.dma_start(out=wq_res[:, 4 * i:4 * i + 4, :],
                                    in_=wq[:, 4 * i:4 * i + 4, :])
            wo_sb = consts.tile([128, GRP, C], BF)
            nc.scalar.dma_start(out=wo_sb, in_=wo.rearrange("(h p) n -> p h n", p=128))
            ot_sb = [otp.tile([128, T], BF, name=f"ot_sb{h}", tag=f"ot{h}", bufs=1)
                     for h in range(GRP)]

            p.update({
                "wk_sb": wk_sb, "wv_sb": wv_sb,
                "cos_sb": cos_sb, "sin_sb": sin_sb, "rot_sb": rot_sb,
                "tri_sb": tri_sb, "ident_f": ident_f, "ot_sb": ot_sb,
                "wq_res": wq_res, "wo_sb": wo_sb,
            })

            if loop_iters is None:
                _emit(nc, tc, dram, p, mybir)
            else:
                with tc.For_i(0, loop_iters, 1) as _i:
                    _emit(nc, tc, dram, p, mybir)

    nc.compile()
    _build_cache[loop_iters] = nc
    return nc


# ---------------------------------------------------------------- host side


def _host_prep(x, Wq, Wk, Wv, Wo):
    import ml_dtypes
    f = np.float32
    bf = ml_dtypes.bfloat16
    scale = f(QK_GAIN) / np.sqrt(f(HD))

    pos = np.arange(T, dtype=f)
    inv_freq = (f(1.0) / (f(10000.0) ** (np.arange(0, ROPE, 2, dtype=f) / f(ROPE)))).astype(f)
    freqs = np.outer(pos, inv_freq).astype(f)            # [T, 16]
    freqs = np.concatenate([freqs, freqs], axis=-1)      # [T, 32]
    cosT = np.ascontiguousarray(np.cos(freqs).astype(f).T)   # [32, T]
    sinT = np.ascontiguousarray(np.sin(freqs).astype(f).T)

    half = ROPE // 2
    Rm = np.zeros((ROPE, ROPE), dtype=f)
    for i in range(half):
        Rm[i, half + i] = -1.0
        Rm[half + i, i] = 1.0
    rotT = np.ascontiguousarray(Rm.T)

    pidx = np.arange(128)[:, None]
    uidx = np.arange(128)[None, :]
    tri = (uidx >= pidx).astype(bf)          # valid iff u >= p
    identf = np.eye(128, dtype=f)

    x = np.asarray(x, dtype=f)
    # [T, C] -> [128, NKT, T]: xt[p, k, t] = x[b][t, k*128+p]
    xTb = [np.ascontiguousarray(x[b].reshape(T, NKT, 128).transpose(2, 1, 0))
           for b in range(B)]

    in_maps = []
    for c in range(N_CORES):
        b, g = divmod(c, GRP)
        in_maps.append({
            "xt": xTb[b],
            "wq": np.ascontiguousarray(
                (Wq[:, 512 * g:512 * (g + 1)] * scale)
                .reshape(NKT, 128, GRP * HD).transpose(1, 0, 2)).astype(f),
            "wk": np.ascontiguousarray(Wk[:, 128 * g:128 * (g + 1)]).astype(f),
            "wv": np.ascontiguousarray(Wv[:, 128 * g:128 * (g + 1)]).astype(f),
            "wo": np.ascontiguousarray(Wo[512 * g:512 * (g + 1), :]).astype(bf),
            "cosd": cosT, "sind": sinT, "rotd": rotT,
            "trid": tri, "identfd": identf,
        })
    return in_maps


def _assemble(z_list):
    out = np.empty((B, T, C), dtype=np.float32)
    for b in range(B):
        acc = np.zeros((T, C), dtype=np.float64)
        for g in range(GRP):
            acc += z_list[b * GRP + g]
        out[b] = acc.astype(np.float32)
    return out


def kernel(x, Wq, Wk, Wv, Wo):
    from concourse.bass_utils import run_bass_kernel_spmd

    nc = _build(None)
    in_maps = _host_prep(x, Wq, Wk, Wv, Wo)
    res = run_bass_kernel_spmd(nc, in_maps, core_ids=list(range(N_CORES)), trace=False)
    return _assemble([res.results[c]["z"] for c in range(N_CORES)])


# ------------------------------------------------------- timing (test harness)


def _make_runner(nc):
    import jax
    from jax.sharding import Mesh, PartitionSpec
    from jax.experimental.shard_map import shard_map
    import concourse.mybir as mybir
    from concourse.bass2jax import _bass_exec_p, install_neuronx_cc_hook, partition_id_tensor

    install_neuronx_cc_hook()
    partition_name = nc.partition_id_tensor.name if nc.partition_id_tensor else None
    in_names, out_names, out_avals = [], [], []
    for alloc in nc.m.functions[0].allocations:
        if not isinstance(alloc, mybir.MemoryLocationSet):
            continue
        name = alloc.memorylocations[0].name
        if alloc.kind == "ExternalInput":
            if name != partition_name:
                in_names.append(name)
        elif alloc.kind == "ExternalOutput":
            out_names.append(name)
            out_avals.append(jax.core.ShapedArray(tuple(alloc.tensor_shape),
                                                  mybir.dt.np(alloc.dtype)))
    n_params = len(in_names)
    all_names = list(in_names) + list(out_names)
    if partition_name is not None:
        all_names.append(partition_name)

    def _body(*args):
        operands = list(args)
        if partition_name is not None:
            operands.append(partition_id_tensor())
        outs = _bass_exec_p.bind(
            *operands,
            out_avals=tuple(out_avals),
            in_names=tuple(all_names),
            out_names=tuple(out_names),
            lowering_input_output_aliases=(),
            sim_require_finite=True,
            sim_require_nnan=True,
            nc=nc,
        )
        return tuple(outs)

    devices = jax.devices()[:N_CORES]
    mesh = Mesh(np.asarray(devices), ("core",))
    n_outs = len(out_names)
    in_specs = (PartitionSpec("core"),) * (n_params + n_outs)
    out_specs = (PartitionSpec("core"),) * n_outs
    fn = jax.jit(shard_map(_body, mesh=mesh, in_specs=in_specs,
                           out_specs=out_specs, check_rep=False))
    return fn, in_names, out_names, out_avals


def _timed_calls(nc, in_maps, n_calls):
    import jax, time
    from jax.sharding import Mesh, PartitionSpec, NamedSharding
    fn, in_names, out_names, out_avals = _make_runner(nc)
    concat = [np.concatenate([np.asarray(in_maps[c][n]) for c in range(N_CORES)], axis=0)
              for n in in_names]
    zeros = [np.zeros((N_CORES * a.shape[0], *a.shape[1:]), a.dtype) for a in out_avals]
    mesh = Mesh(np.asarray(jax.devices()[:N_CORES]), ("core",))
    shd = NamedSharding(mesh, PartitionSpec("core"))
    args = [jax.device_put(a, shd) for a in concat + zeros]
    out = fn(*args)
    jax.block_until_ready(out)
    ts = []
    for _ in range(n_calls):
        t0 = time.time()
        out = fn(*args)
        jax.block_until_ready(out)
        ts.append(time.time() - t0)
    z_list = [np.asarray(out[0]).reshape(N_CORES, T, C)[c] for c in range(N_CORES)]
    return np.array(ts), z_list


def _robust_min(ts):
    ts = np.sort(np.asarray(ts))
    # guard against rare fast outliers (axon timing artifacts): take the
    # median of the 3 smallest plausible values
    lo = ts[ts >= np.median(ts) * 0.8]
    return lo[:3].mean() if len(lo) >= 3 else ts.min()


def run_and_measure(inputs, iters=40, n_calls=40):
    """Returns (output, hw_time_ns, ts1, tsk). K=1 build gives correctness;
    For_i(iters) build gives timing: (T_k - T_1)/(iters-1)."""
    in_maps = _host_prep(**inputs)
    nc1 = _build(None)
    ts1, z_list = _timed_calls(nc1, in_maps, n_calls)
    out = _assemble(z_list)
    nck = _build(iters)
    tsk, _ = _timed_calls(nck, in_maps, n_calls)
    hw_ns = (_robust_min(tsk) - _robust_min(ts1)) / (iters - 1) * 1e9
    return out, hw_ns, ts1, tsk
